# revision 16
# baseline (speedup 1.0000x reference)
"""Trainium2 Bass kernel for Block (2D overlapping patch extraction).

x: [4, 8, 512, 512] f32 -> out: [4, 8, 3969, 16, 16]
block 16x16, stride 8x8, 'valid' -> nbh = nbw = 63.

Sharding: data-parallel over the 32 (batch*channel) images; 4 images per
NeuronCore across 8 cores. No cross-core communication.

Per-core plan (images c in [0,4), block rows i in [0,63), two chunks of 32
block rows overlapping at i=31, partition p = ii*4 + c):
 - Load (gpsimd queue): L[p] = 8-row group i0+ii of image c (16KB/partition,
   2MB/chunk) + a tiny 4-partition halo H = group i0+32.
 - TensorE: shifted-identity fp32 matmuls produce PSUM[p] = L[p+4] (the a=1
   row-group) -- partition shifts are illegal for engine APs (start must be
   0 mod 32) but free in a matmul, and fp32 identity-matmul passthrough is
   bit-exact (verified on HW).  PSUM bank k holds column group
   [64k, 64k+64) of all 8 rows so consumers unblock bank by bank.
 - VectorE copies assemble the output layout per j-group:
   out[p][j, a*8+h, b*8+w] = in[8*(i0+ii+a)+h, 8*(j+b)+w]; a=0 reads L,
   a=1 reads PSUM (split at 64-col bank boundaries).
 - Stores (sync/scalar HWDGE queues): one DMA per j-group, 63KB/partition
   total, fired as soon as its quarter of O is complete.

DMA lore this kernel is built around (all measured on HW):
 - The DRAM-side AP's outermost dim count sets the SDMA engine spray:
   multiples of 16 engage all 16 engines (~400GB/s); 4 or 31 pin the DMA to
   1-4 engines.
 - Descriptors (per-partition contiguous runs) must be <=16KB; 32KB+ runs
   at half the per-engine rate.
 - Loads and stores must sit on different queues: a store in the gpsimd
   FIFO waits on copies and head-of-line-blocks later loads.
"""

import numpy as np

NCORES = 8
CH_PER_CORE = 4
H = W = 512
BH = BW = 16
SH = SW = 8
NB = 63          # blocks per axis
ROW = W          # elements per image row
IMG = H * W      # elements per image
OUT_BLK = BH * BW            # 256 elements per block
OUT_ROWCH = NB * OUT_BLK     # 16128 elements per block-row
OUT_IMG = NB * NB * OUT_BLK  # elements per output image

_CACHE = {}


def _shift_mats():
    s4 = np.zeros((128, 128), np.float32)
    for m in range(124):
        s4[m + 4, m] = 1.0
    sh = np.zeros((4, 128), np.float32)
    for k in range(4):
        sh[k, k + 124] = 1.0
    return s4, sh


def _build_nc():
    import concourse.bass as bass
    import concourse.bacc as bacc
    import concourse.mybir as mybir
    from concourse import tile

    f32 = mybir.dt.float32
    nc = bacc.Bacc(
        "TRN2", target_bir_lowering=False, debug=False, num_devices=NCORES
    )
    xs = nc.dram_tensor("xs", [CH_PER_CORE, H, W], f32, kind="ExternalInput")
    s4d = nc.dram_tensor("s4", [128, 128], f32, kind="ExternalInput")
    shd = nc.dram_tensor("sh", [4, 128], f32, kind="ExternalInput")
    out = nc.dram_tensor(
        "out", [CH_PER_CORE, NB * NB, BH, BW], f32, kind="ExternalOutput"
    )

    GRP = SH * ROW  # 4096 elements: one 8-row group
    J_GROUPS = ((0, 16), (16, 16), (32, 16), (48, 15))
    STORE_ENG = {
        (0, 0): "sync", (0, 1): "scalar", (0, 2): "sync", (0, 3): "scalar",
        (31, 0): "scalar", (31, 1): "sync", (31, 2): "scalar", (31, 3): "sync",
    }
    nI = 32
    P = CH_PER_CORE * nI  # 128

    with tile.TileContext(nc) as tc:
        with (
            tc.tile_pool(name="cp", bufs=1) as cpool,
            tc.tile_pool(name="lp", bufs=1) as lp,
            tc.tile_pool(name="op", bufs=2) as op,
            tc.tile_pool(name="pp", bufs=1, space="PSUM") as pp,
        ):
            s4t = cpool.tile([128, 128], f32, name="s4t", tag="s4t")
            sht = cpool.tile([4, 128], f32, name="sht", tag="sht")
            nc.gpsimd.dma_start(out=s4t[:, :], in_=s4d.ap())
            nc.gpsimd.dma_start(out=sht[:, :], in_=shd.ap())

            # All loads first: they own the gpsimd queue end to end.
            Ls, Hs = {}, {}
            for i0 in (0, 31):
                L = lp.tile([128, GRP], f32, name=f"L{i0}", tag=f"L{i0}")
                nc.gpsimd.dma_start(
                    out=L[:, :],
                    in_=bass.AP(
                        xs, i0 * GRP, [[GRP, nI], [IMG, CH_PER_CORE], [1, GRP]]
                    ),
                )
                HL = lp.tile([4, GRP], f32, name=f"H{i0}", tag=f"H{i0}")
                nc.gpsimd.dma_start(
                    out=HL[:, :],
                    in_=bass.AP(xs, (i0 + 32) * GRP, [[IMG, CH_PER_CORE], [1, GRP]]),
                )
                Ls[i0], Hs[i0] = L, HL

            for i0 in (0, 31):
                L, HL = Ls[i0], Hs[i0]
                L3 = L.rearrange("p (h c) -> p h c", h=SH, c=ROW)
                H3 = HL.rearrange("p (h c) -> p h c", h=SH, c=ROW)

                # PSUM bank k <- column group [64k, 64k+64) of the +1-group
                # shift: PSB[k][m, h*64+c] = L[m+4][h, 64k+c] (m<124), halo
                # rows accumulated on top for m>=124.
                PSB = []
                for k in range(8):
                    ps = pp.tile([128, 512], f32, name=f"ps{i0}_{k}", tag=f"ps{k}")
                    nc.tensor.matmul(
                        ps[:, :], s4t[:, :], L3[:, :, 64 * k : 64 * k + 64],
                        start=True, stop=False,
                    )
                    nc.tensor.matmul(
                        ps[:, :], sht[:, :], H3[:, :, 64 * k : 64 * k + 64],
                        start=False, stop=True,
                    )
                    PSB.append(ps)

                O = op.tile([128, OUT_ROWCH], f32, name=f"O{i0}", tag="O")
                O_r = O.rearrange(
                    "p (j A h B w) -> p A B j h w", j=NB, A=2, h=SH, B=2, w=SW
                )

                for gi, (j0, njg) in enumerate(J_GROUPS):
                    for b in (0, 1):
                        # a=0 straight from L.
                        c0 = SW * (j0 + b)
                        nc.vector.tensor_copy(
                            out=O_r[:, 0, b, j0 : j0 + njg],
                            in_=L3[:, :, c0 : c0 + SW * njg].rearrange(
                                "p h (j w) -> p j h w", w=SW
                            ),
                        )
                        # a=1 from PSUM, split at 64-column bank boundaries:
                        # j's bank k = (j + b + j0)*8 // 64.
                        s = 0
                        while s < njg:
                            k = (j0 + s + b) * SW // 64
                            e = min(njg, (k + 1) * 8 - (j0 + b))
                            ps = PSB[k]
                            ps3 = ps.rearrange("p (h c) -> p h c", h=SH, c=64)
                            lc0 = SW * (j0 + s + b) - 64 * k
                            nc.vector.tensor_copy(
                                out=O_r[:, 1, b, j0 + s : j0 + e],
                                in_=ps3[:, :, lc0 : lc0 + SW * (e - s)].rearrange(
                                    "p h (j w) -> p j h w", w=SW
                                ),
                            )
                            s = e
                    eng = getattr(nc, STORE_ENG[(i0, gi)])
                    eng.dma_start(
                        out=bass.AP(
                            out,
                            i0 * OUT_ROWCH + j0 * OUT_BLK,
                            [
                                [OUT_ROWCH, nI],
                                [OUT_IMG, CH_PER_CORE],
                                [1, njg * OUT_BLK],
                            ],
                        ),
                        in_=O[:P, j0 * OUT_BLK : (j0 + njg) * OUT_BLK],
                    )
    nc.compile()
    return nc


def get_nc():
    if "nc" not in _CACHE:
        _CACHE["nc"] = _build_nc()
    return _CACHE["nc"]


def _enable_jax_compile_cache():
    try:
        import jax

        jax.config.update("jax_compilation_cache_dir", "/tmp/jax_neff_cache")
        jax.config.update("jax_persistent_cache_min_entry_size_bytes", -1)
        jax.config.update("jax_persistent_cache_min_compile_time_secs", 0.0)
    except Exception:
        pass


def run_spmd(in_maps, **kwargs):
    from concourse.bass_utils import run_bass_kernel_spmd

    _enable_jax_compile_cache()
    return run_bass_kernel_spmd(
        get_nc(), in_maps, core_ids=list(range(NCORES)), **kwargs
    )


def make_in_maps(x: np.ndarray):
    xs = np.asarray(x, dtype=np.float32).reshape(-1, H, W)
    s4, sh = _shift_mats()
    return [
        {
            "xs": np.ascontiguousarray(xs[c * CH_PER_CORE : (c + 1) * CH_PER_CORE]),
            "s4": s4,
            "sh": sh,
        }
        for c in range(NCORES)
    ]


def assemble(results, batch_shape):
    outs = np.stack([r["out"] for r in results])  # [8, 4, 3969, 16, 16]
    return outs.reshape(*batch_shape, NB * NB, BH, BW)


def kernel(**inputs) -> np.ndarray:
    x = np.asarray(inputs["x"])
    res = run_spmd(make_in_maps(x))
    return assemble(res.results, x.shape[:2])
